# revision 39
# baseline (speedup 1.0000x reference)
"""PiLoraLayer TRN2 kernel: y = x + (alpha/r) * sin((2/pi) * (x @ A) @ B).

x: [4, 4096, 4096] f32; A = A_int8 * scale_A (per-col), B = B_int8 * scale_B
(per-col); rank 16 bottleneck.

Strategy (data-parallel over 8 NeuronCores, fully transposed dataflow):
- Host: xh = (x/2) as fp16 (residual+matmul source), A2 = 2*A_int8 as fp16,
  Bp = scale_A[:,None] * B_q * scale_B[None,:] / pi^2 (f32). Then
  u = (xh @ A2) @ Bp = arg/(2*pi) and y = 2*(xh + sin(2*pi*frac(u))).
- Shard x's 16384 token rows into 8 x [2048, 4096] shards, one per core.
- Device (per core), per super-tile of T tokens, ALL in transposed space:
    - ONE dma_start_transpose loads xT slab [128h, KC, T] (xbar HW transpose)
    - mm1: h1T[16, T] = sum_k A2_k.T @ xT_k (PSUM accumulate, 32 chunks)
    - mm2 (transposed): uT chunk [128h, T] = Bp_chunk.T @ h1T -- Bp slices
      are natural stationaries, no transpose of anything needed (f32r for
      precision; fp16 Bp would add ~5e-3 rel err).
    - Range reduction per 1024-elem block: kq = (u + 1.5*2^23) - 1.5*2^23
      (one DVE tensor_scalar, RNE-to-integer, fp16 exact for |k|<=2048);
      PE accumulates -kq via fp16 negative-identity matmul -> frac in PSUM;
      ACT: s = sin(2*pi*frac) -> fp16.
    - DVE fp16 TT add (2x mode): slab += s  (slab holds xh.T = x.T/2)
    - DMA slab chunk out as y.T/2; host returns 2 * y_t.T.
- No PE transposes (HAM-blind + PSUM copies), no ACT copies, single x read,
  fp16 I/O: 3x less DMA bytes and ~2x less engine work than the v1 kernel.
"""

import os
import sys

sys.path.insert(0, "/opt/trn_rl_repo")

import numpy as np

HOST_T = bool(os.environ.get("HOST_T"))  # debug: host-side x transpose
DBG = os.environ.get("DBG", "")  # debug: "echo" = skip add, "sin" = only sin part
YW = os.environ.get("YW", "gpsimd")  # y-write engine: gpsimd (SWDGE) | act (HWDGE)
ROUND_ENG = os.environ.get("ROUND", "vector")  # round engine (GPSIMD can't read PSUM)
ADD_ENG = os.environ.get("ADD", "gpsimd")  # residual-add engine: vector | gpsimd

import concourse.bacc as bacc
import concourse.tile as tile
from concourse import mybir
from concourse.bass import ts
from concourse.bass_utils import run_bass_kernel_spmd

P = 128
HIDDEN = 4096
RANK = 16
N_CORES = 8
TOTAL_ROWS = 4 * 4096
ROWS = TOTAL_ROWS // N_CORES  # 2048 per core
SUPER = 512  # tokens per steady-state super-tile
KC = HIDDEN // P  # 32 hidden chunks
BLK = 1024  # tail block free elems (2 PSUM banks)
MAGIC = 12582912.0  # 1.5 * 2^23: f32 add/sub rounds to nearest integer
SCALE_2PI = 6.283185  # slightly < 2*pi so the LUT arg stays inside [-pi, pi]

F32 = mybir.dt.float32
F32R = mybir.dt.float32r
F16 = mybir.dt.float16


def build_nc(rows: int = ROWS):
    """Build the per-core Bass program for a [rows, 4096] token shard."""
    nc = bacc.Bacc(
        "TRN2",
        target_bir_lowering=False,
        debug=False,
        enable_asserts=False,
        num_devices=N_CORES,
    )
    if HOST_T:
        x_d = nc.dram_tensor("x", [HIDDEN, rows], F16, kind="ExternalInput").ap()
        x_t = x_d.rearrange("(k p) t -> p k t", p=P)
    else:
        x_d = nc.dram_tensor("x", [rows, HIDDEN], F16, kind="ExternalInput").ap()
    a_d = nc.dram_tensor("A", [HIDDEN, RANK], F16, kind="ExternalInput").ap()
    bp_d = nc.dram_tensor("Bp", [RANK, HIDDEN], F32, kind="ExternalInput").ap()
    rep_d = nc.dram_tensor("REP", [RANK, P], F32, kind="ExternalInput").ap()
    y_d = nc.dram_tensor("out", [HIDDEN, rows], F16, kind="ExternalOutput").ap()
    y_r = y_d.rearrange("(k p) t -> p k t", p=P)  # [128, KC, rows]

    # uniform super-tiles: PSUM matmul outputs must stay bank-aligned, which
    # holds only for T=512 (chunk writes at 0/512 f32 offsets)
    assert rows % SUPER == 0
    layout = [(i * SUPER, SUPER) for i in range(rows // SUPER)]

    with tile.TileContext(nc) as tc:
        with (
            tc.tile_pool(name="singles", bufs=1) as singles,
            tc.tile_pool(name="slabp", bufs=3) as slab_pool,
            tc.tile_pool(name="kp", bufs=5) as kpool,
            tc.tile_pool(name="sp", bufs=6) as spool,
            tc.tile_pool(name="h1sb", bufs=2) as h1pool,
            tc.tile_pool(name="h1p", bufs=1, space="PSUM") as h1_psum,
            tc.tile_pool(name="up", bufs=3, space="PSUM") as u_psum,
        ):
            nident = singles.tile([P, P], F16)
            nc.gpsimd.memset(nident[:], 0.0)
            nc.gpsimd.affine_select(
                out=nident[:],
                in_=nident[:],
                compare_op=mybir.AluOpType.not_equal,
                fill=-1.0,
                base=0,
                pattern=[[-1, P]],
                channel_multiplier=1,
            )
            a_sb = singles.tile([P, KC, RANK], F16)
            nc.sync.dma_start(
                out=a_sb[:], in_=a_d.rearrange("(k p) r -> p k r", p=P)
            )
            # NOTE: do NOT dma with .bitcast(F32R) in a program that also uses
            # dma_start_transpose -- the f32r-tagged DMA descriptor poisons the
            # xbar path and the transposed f16 data comes back f32r-rounded
            # (even 16-bit lanes quantized). Load plain f32, then produce the
            # f32r copy on-chip with DVE (a legal f32r producer for the PE).
            # bp4[32i : 32i+16, g, :] = Bp[:, (4g+i)*128 : (4g+i+1)*128] --
            # the 4 partition groups let 4 rank-16 fill matmuls run
            # concurrently in distinct PE row groups (tile_position).
            bp4_f32 = singles.tile([P, KC // 4, P], F32)
            bp_r4 = bp_d.rearrange("r (g i c) -> r g i c", g=KC // 4, i=4)
            for i in range(4):
                nc.sync.dma_start(
                    out=bp4_f32[32 * i : 32 * i + RANK, :, :],
                    in_=bp_r4[:, :, i, :],
                )
            bp4 = singles.tile([P, KC // 4, P], F32R)
            for i in range(4):
                nc.vector.tensor_copy(
                    bp4[32 * i : 32 * i + RANK, :, :],
                    bp4_f32[32 * i : 32 * i + RANK, :, :],
                )
            # REP[r, 32i+r] = 1: one PE matmul replicates h1T into all four
            # 32-partition groups so packed fills see matching partition bases
            rep_f32 = singles.tile([RANK, P], F32)
            nc.sync.dma_start(out=rep_f32[:], in_=rep_d[:, :])
            rep_sb = singles.tile([RANK, P], F32R)
            nc.vector.tensor_copy(rep_sb[:], rep_f32[:])

            round_eng = nc.gpsimd if ROUND_ENG == "gpsimd" else nc.vector
            WG = 4  # tail blocks per y-write DMA

            def emit_tail(state):
                """Tail for a finished super-tile, software-pipelined:
                FILL runs one block ahead of SUB so the PE never stalls on
                the round; y-writes batch WG blocks and trail the adds."""
                slab, h1_sb, t0, T = state
                cb = BLK // T  # hidden chunks per block
                nb = KC // cb  # blocks per super-tile
                ydma = nc.scalar.dma_start if YW == "act" else nc.gpsimd.dma_start

                def emit_write(g):
                    lo, hi = g * WG * cb, (g + 1) * WG * cb
                    ydma(
                        out=y_r[:, lo:hi, t0 : t0 + T],
                        in_=slab[:, lo:hi, :],
                    )

                if DBG in ("pure", "mm1"):
                    for g in range(nb // WG):
                        emit_write(g)
                    return

                u_tiles = {}

                def emit_pack(g):
                    """4 rank-16 fill matmuls packed into distinct PE row
                    groups -- they execute concurrently (blocks 2g, 2g+1)."""
                    for m in (2 * g, 2 * g + 1):
                        u_tiles[m] = u_psum.tile([P, cb, T], F32, name="u_ps")
                    for i in range(4):
                        m, c = 2 * g + i // cb, i % cb
                        nc.tensor.matmul(
                            u_tiles[m][:, c, :],
                            bp4[32 * i : 32 * i + RANK, g, :],
                            h1_sb[32 * i : 32 * i + RANK, :],
                            start=True,
                            stop=True,
                            tile_position=(32 * i, 0),
                        )

                def emit_add(m, s):
                    """Residual add for block m (deferred 2 blocks so the DVE
                    queue never stalls on sin), then the batched y-write."""
                    if DBG == "echo":
                        pass
                    elif DBG == "sin":
                        nc.vector.tensor_copy(slab[:, m * cb : (m + 1) * cb, :], s[:])
                    else:
                        add_eng = nc.gpsimd if ADD_ENG == "gpsimd" else nc.vector
                        add_eng.tensor_tensor(
                            slab[:, m * cb : (m + 1) * cb, :],
                            slab[:, m * cb : (m + 1) * cb, :],
                            s[:],
                            mybir.AluOpType.add,
                        )
                    if (m + 1) % WG == 0:
                        emit_write((m + 1) // WG - 1)

                emit_pack(0)
                s_tiles = {}
                for n in range(nb):
                    if n % 2 == 1 and n + 2 < nb:
                        emit_pack((n + 1) // 2)
                    u_ps = u_tiles.pop(n)
                    kq = kpool.tile([P, cb, T], F16)
                    round_eng.tensor_scalar(
                        kq[:],
                        u_ps[:],
                        MAGIC,
                        MAGIC,
                        mybir.AluOpType.add,
                        mybir.AluOpType.subtract,
                    )
                    for c in range(cb):
                        nc.tensor.matmul(
                            u_ps[:, c, :],
                            nident[:],
                            kq[:, c, :],
                            start=False,
                            stop=True,
                            skip_group_check=True,
                        )
                    s = spool.tile([P, cb, T], F16)
                    nc.scalar.activation(
                        out=s[:],
                        in_=u_ps[:],
                        func=mybir.ActivationFunctionType.Sin,
                        scale=SCALE_2PI,
                    )
                    s_tiles[n] = s
                    if n >= 2:
                        emit_add(n - 2, s_tiles.pop(n - 2))
                for m in (nb - 2, nb - 1):
                    emit_add(m, s_tiles.pop(m))

            prev = None
            for st, (t0, T) in enumerate(layout):
                slab = slab_pool.tile([P, KC, T], F16)
                if HOST_T:
                    nc.sync.dma_start(out=slab[:], in_=x_t[:, :, t0 : t0 + T])
                else:
                    nc.sync.dma_start_transpose(out=slab[:], in_=x_d[t0 : t0 + T, :])
                if prev is not None:
                    emit_tail(prev)
                h1_4 = None
                if DBG != "pure":
                    h1_ps = h1_psum.tile([RANK, T], F32, name="h1_ps")
                    for k in range(KC):
                        nc.tensor.matmul(
                            h1_ps[:],
                            a_sb[:, k, :],
                            slab[:, k, :],
                            start=(k == 0),
                            stop=(k == KC - 1),
                        )
                    h1_sb = h1pool.tile([RANK, T], F32R, name="h1_sb")
                    nc.vector.tensor_copy(h1_sb[:], h1_ps[:])
                    # replicate h1T into all 4 partition groups for the
                    # row-group-packed fill matmuls
                    h1_4ps = h1_psum.tile([P, T], F32, name="h1_4ps")
                    nc.tensor.matmul(
                        h1_4ps[:], rep_sb[:], h1_sb[:], start=True, stop=True
                    )
                    h1_4 = h1pool.tile([P, T], F32R, name="h1_4")
                    nc.vector.tensor_copy(h1_4[:], h1_4ps[:])
                prev = (slab, h1_4, t0, T)

            emit_tail(prev)

    nc.compile()
    return nc


_NC_CACHE: dict[int, object] = {}


def _get_nc(rows: int = ROWS):
    nc = _NC_CACHE.get(rows)
    if nc is None:
        nc = build_nc(rows)
        _NC_CACHE[rows] = nc
    return nc


def _prep_weights(A_int8, B_int8, scale_A, scale_B):
    a2 = np.ascontiguousarray((A_int8.astype(np.float32) * 2.0).astype(np.float16))
    bp = np.ascontiguousarray(
        scale_A.astype(np.float32)[:, None]
        * B_int8.astype(np.float32)
        * scale_B.astype(np.float32)[None, :]
        * np.float32(1.0 / (np.pi * np.pi))
    )
    rep = np.zeros((RANK, P), dtype=np.float32)
    for i in range(4):
        rep[np.arange(RANK), 32 * i + np.arange(RANK)] = 1.0
    return a2, bp, rep


def _prep_in_maps(x, A_int8, B_int8, scale_A, scale_B, rows=ROWS, n_cores=N_CORES):
    xf = x.reshape(-1, HIDDEN)
    a2, bp, rep = _prep_weights(A_int8, B_int8, scale_A, scale_B)
    def shard(i):
        xh = (xf[i * rows : (i + 1) * rows] * np.float32(0.5)).astype(np.float16)
        if HOST_T:
            xh = xh.T
        return np.ascontiguousarray(xh)

    return [
        {"x": shard(i), "A": a2, "Bp": bp, "REP": rep}
        for i in range(n_cores)
    ]


def _postprocess(results, orig_shape):
    parts = [
        (np.ascontiguousarray(r["out"].T).astype(np.float32) * np.float32(2.0))
        for r in results
    ]
    return np.concatenate(parts, axis=0).reshape(orig_shape)


def kernel(x, A_int8, B_int8, scale_A, scale_B):
    x = np.asarray(x)
    orig_shape = x.shape
    in_maps = _prep_in_maps(
        np.ascontiguousarray(x.reshape(TOTAL_ROWS, HIDDEN)),
        np.asarray(A_int8),
        np.asarray(B_int8),
        np.asarray(scale_A),
        np.asarray(scale_B),
    )
    nc = _get_nc(ROWS)
    res = run_bass_kernel_spmd(nc, in_maps, core_ids=list(range(N_CORES)))
    return _postprocess(res.results, orig_shape)
